# revision 25
# baseline (speedup 1.0000x reference)
"""Trainium2 Bass kernel for masked-softmax attention scoring.

Reference computation (B=128, T=512, K=1024, Q=1024):
    mids  = einsum("kq,bq->bk", W, query)
    s     = tanh(einsum("btk,bk->bt", key, mids) + bias)
    attn  = softmax-like: exp(s - max) * mask / sum(exp(s - max) * mask)

The max-subtraction cancels exactly in the ratio (tanh is bounded), so the
device computes  attn = exp(tanh(.)) * mask / sum_t(exp(tanh(.)) * mask).

Sharding: data-parallel over B across 8 NeuronCores (16 batches/core).

v8 design ("key through the PE weight port", fp16 wire format):
  * All large operands stream as fp16 (key 16 MB, W^T 2 MB per core),
    halving HBM traffic vs fp32.  Raw scores have std ~59 and tanh
    saturates hard, so fp16 rounding (~0.13 abs err on scores) perturbs
    only the ~3% of entries with |s| < 3: measured rel-l2 1.39e-3 vs the
    2e-2 budget.  (fp8-e4m3 key was tried and fails: rel-l2 4.5e-2 at 2
    of 8 chunks -- tanh-window sign flips scale superlinearly.)
  * Host pre-transposes key to k-on-partitions layout; every dma_start
    reads a fully contiguous DRAM region (dense 4-8 KB per-partition runs).
  * mids^T = W @ query computed directly in k-on-partitions layout:
    per (qc, kc) chunk, lhsT = W^T chunk [q, k] (128-col fp16 stationary,
    fast-weight-load), rhs = query^T chunk [q, b] -> mids_ps[k, (kc, b)].
  * scores: per (kc, b, tb) the key chunk [kp, 128 t] is the 128-col fp16
    stationary (FWL, ~30 ns/load measured warm) and mids^T[:, kc, b] is a
    1-column moving operand -> out [t(128 partitions), (tb, b)] accumulated
    over kc in ONE PSUM bank.  No diagonal extraction, trivial PSUM
    footprint; PE sits well under the DMA roofline.
  * DMA model (from perfetto): both trigger engines (SP + Activation HWDGE)
    feed the SAME 16 HW queues in global FIFO enqueue order at ~360-380
    GB/s aggregate, and each dma_start costs ~0.6-1 us of descriptor
    generation on its issuing sequencer.  So enqueue order must equal
    consumption order: W halves first (one per sequencer), slab half-pieces
    pairwise adjacent, tiny epilogue loads (bias/mask) at the very end.
    The last two slabs are quartered to shorten the post-last-byte PE tail.
  * epilogue: tanh/exp straight out of PSUM on ScalarE, fp16 mask multiply
    on DVE, row sums over the t partition dim via 4 accumulating
    ones-vector matmuls, reciprocal, partition-broadcast of 1/denom via a
    rank-1 fp16 ones matmul, final scale, DMA out.  PSUM accumulation uses
    a single start=True on the first matmul per bank (bank-fresh semantics
    make per-column accumulation groups safe).
"""

import sys

if "/opt/trn_rl_repo" not in sys.path:
    sys.path.insert(0, "/opt/trn_rl_repo")

from contextlib import ExitStack

import numpy as np

# ---- problem constants (hardcoded per spec) ----
B, T, K, Q = 128, 512, 1024, 1024
NCORES = 8
BS = B // NCORES          # 16 batches per core
P = 128                   # SBUF partitions
KC = K // P               # 8 contraction chunks for the scores matmuls
QC = Q // P               # 8 contraction chunks for the mids matmuls
TB = T // P               # 4 t-blocks of 128 (PSUM/output partition dim)
N_FP8 = 0                 # fp8-e4m3 key chunks: measured rel-l2 4.5e-2 at 2
                          # chunks (superlinear tanh-window sign flips) -- the
                          # 2e-2 gate forces all-fp16 key

N_F16 = KC - N_FP8
NQT = 2                   # last NQT fp16 slabs are quartered (short PE tail)
KEY_BUFS = 6              # fp16 half-slab pool depth per piece tag

# slab issue order: logical kc by arrival position; fp8 slabs (N_FP8 of
# them, at the end of the logical range) interleave early/mid-stream, the
# quartered fp16 slabs go last.
_ORDER = [0, 6, 1, 2, 7, 3, 4, 5] if N_FP8 == 2 else (
    [0, 7, 1, 2, 3, 4, 5, 6] if N_FP8 == 1 else list(range(KC))
)

_STATE: dict = {}


def _build_nc():
    import concourse.tile as tile
    from concourse import bacc, mybir

    f32 = mybir.dt.float32
    f16 = mybir.dt.float16
    f8 = mybir.dt.float8e4
    nc = bacc.Bacc()

    # fp16 key chunks: first N_F16-NQT half-split, last NQT quarter-split
    kh_e = nc.declare_dram_parameter(
        "keyh", [N_F16 - NQT, 2, P, BS // 2, T], f16, isOutput=False
    )
    kq_e = nc.declare_dram_parameter(
        "keyq", [NQT, 4, P, BS // 4, T], f16, isOutput=False
    )
    k8_e = (
        nc.declare_dram_parameter("key8", [N_FP8, 2, P, BS // 2, T], f8, isOutput=False)
        if N_FP8
        else None
    )
    # wt[h, qp, qh, kc, kl] = W[kc*128 + kl, (h*4 + qh)*128 + qp]
    wt_e = nc.declare_dram_parameter(
        "wt", [2, P, QC // 2, KC, P], f16, isOutput=False
    )
    qt_e = nc.declare_dram_parameter("qt", [P, QC, BS], f16, isOutput=False)
    maskr_e = nc.declare_dram_parameter("maskr", [P, TB, BS], f16, isOutput=False)
    bias_e = nc.declare_dram_parameter("biasb", [P, 1], f32, isOutput=False)
    out_e = nc.declare_dram_parameter("out", [2, P, TB, BS // 2], f32, isOutput=True)

    with tile.TileContext(nc) as tc, ExitStack() as ctx:
        const = ctx.enter_context(tc.tile_pool(name="const", bufs=1))
        kpool = ctx.enter_context(tc.tile_pool(name="key", bufs=KEY_BUFS))
        qpool = ctx.enter_context(tc.tile_pool(name="keyq", bufs=2))
        psum = ctx.enter_context(tc.tile_pool(name="psum", bufs=1, space="PSUM"))

        # All dma_starts from both trigger engines land in the SAME 16 HW
        # queues in global enqueue (FIFO) order, and each dma_start costs
        # ~0.6-1 us of descriptor generation on its issuing sequencer.  So
        # enqueue order must match consumption order: both W halves first
        # (one per sequencer), then key slab pieces pairwise, tiny loads
        # late.
        wt_sbs = [
            const.tile([P, QC // 2, KC, P], f16, tag=f"wt{h}", name=f"wt{h}")
            for h in range(2)
        ]
        bias_sb = const.tile([P, 1], f32)
        qt_sb = const.tile([P, QC, BS], f16)
        mask_sb2 = const.tile([P, TB, BS], f16, tag="mask", name="mask_sb2")
        nc.sync.dma_start(out=wt_sbs[0][:], in_=wt_e[0])
        nc.scalar.dma_start(out=qt_sb[:], in_=qt_e[:])
        nc.scalar.dma_start(out=wt_sbs[1][:], in_=wt_e[1])

        ones_col = const.tile([P, 1], f16)
        nc.vector.memset(ones_col[:], 1.0)
        ones_row = const.tile([1, P], f16)
        nc.vector.memset(ones_row[:], 1.0)

        rings = [nc.sync, nc.scalar]

        # Every DMA piece gets its own tile; pieces of one slab go out on
        # both trigger engines so their descriptors enqueue adjacently.
        _state = {"nh": 0, "nq": 0}

        def issue_slab_dmas(kc):
            if kc >= N_F16 - NQT:
                pieces, w = 4, BS // 4
                tiles = []
                for pc in range(pieces):
                    t = qpool.tile([P, w, T], f16, tag=f"ktq{pc}", name=f"ktq{pc}")
                    rings[pc % 2].dma_start(out=t[:], in_=kq_e[_state["nq"], pc])
                    tiles.append(t)
                _state["nq"] += 1
            else:
                pieces, w = 2, BS // 2
                tiles = []
                for pc in range(pieces):
                    t = kpool.tile([P, w, T], f16, tag=f"kth{pc}", name=f"kth{pc}")
                    rings[pc % 2].dma_start(out=t[:], in_=kh_e[_state["nh"], pc])
                    tiles.append(t)
                _state["nh"] += 1
            return tiles, w

        # slab 0's pieces enqueue right behind the W halves, BEFORE the
        # mids matmuls are issued (reads must follow writes in issue order)
        slab_q = [issue_slab_dmas(_ORDER[0])]

        # ---- mids^T[k, (kc, b)] = sum_q W[k, q] query[b, q] ----
        # h-major order: the wtA-half matmuls stream while wtB still loads.
        # Single start=True into the fresh bank.
        mids_ps = psum.tile([P, KC, BS], f32)
        for qi, (h, qh) in enumerate(
            [(0, 0), (0, 1), (0, 2), (0, 3), (1, 0), (1, 1), (1, 2), (1, 3)]
        ):
            for kc in range(KC):
                nc.tensor.matmul(
                    mids_ps[:, kc, :],
                    lhsT=wt_sbs[h][:, qh, kc, :],
                    rhs=qt_sb[:, h * (QC // 2) + qh, :],
                    start=(qi == 0 and kc == 0),
                    stop=(qi == QC - 1),
                )
        mids_sb = const.tile([P, KC, BS], f16)
        nc.vector.tensor_copy(mids_sb[:], mids_ps[:])

        # ---- scores[t + 128 tb, (tb, b)] += key-chunk^T @ mids-col ----
        # Two PSUM banks split by batch half so the b0-7 epilogue overlaps
        # the PE finishing b8-15 (different banks -> no PSUM collision).
        HB = BS // 2
        scores_pss = [
            psum.tile([P, TB, HB], f32, tag=f"sc{h}", name=f"sc{h}")
            for h in range(2)
        ]
        for pos, kc in enumerate(_ORDER):
            if pos + 1 < KC:
                slab_q.append(issue_slab_dmas(_ORDER[pos + 1]))
            tiles, w = slab_q[pos]
            for b in range(BS):
                kt = tiles[b // w]
                for tb in range(TB):
                    nc.tensor.matmul(
                        scores_pss[b // HB][:, tb, b % HB : b % HB + 1],
                        lhsT=kt[:, b % w, tb * P : (tb + 1) * P],
                        rhs=mids_sb[:, kc, b : b + 1],
                        start=(pos == 0 and b % HB == 0 and tb == 0),
                        stop=(pos == KC - 1),
                    )

        # epilogue-only loads: enqueued after every key piece, data lands
        # well before the epilogue needs it
        nc.scalar.dma_start(out=bias_sb[:], in_=bias_e[:])
        nc.scalar.dma_start(out=mask_sb2[:], in_=maskr_e[:])

        # ---- epilogue: per batch-half pipeline: tanh/exp (ScalarE, straight
        # from PSUM), fp16 mask mult (DVE), rowsums over t partitions via
        # accumulating ones matmuls, reciprocal, rank-1 broadcast, scale,
        # split DMA out.  Half A runs while the PE still finishes half B.
        for h in range(2):
            sl = slice(h * HB, (h + 1) * HB)
            tanh_h = const.tile([P, TB, HB], f32, tag=f"tanh{h}", name=f"tanh{h}")
            nc.scalar.activation(
                out=tanh_h[:],
                in_=scores_pss[h][:],
                func=mybir.ActivationFunctionType.Tanh,
                bias=bias_sb[:],
                scale=1.0,
            )
            exp_h = const.tile([P, TB, HB], f16, tag=f"exp{h}", name=f"exp{h}")
            nc.scalar.activation(
                out=exp_h[:], in_=tanh_h[:], func=mybir.ActivationFunctionType.Exp
            )
            em_h = const.tile([P, TB, HB], f16, tag=f"em{h}", name=f"em{h}")
            nc.vector.tensor_tensor(
                em_h[:], exp_h[:], mask_sb2[:, :, sl], mybir.AluOpType.mult
            )
            sums_h = psum.tile([1, HB], f32, tag=f"sums{h}", name=f"sums{h}")
            for tb in range(TB):
                nc.tensor.matmul(
                    sums_h[:],
                    lhsT=ones_col[:],
                    rhs=em_h[:, tb, :],
                    start=(tb == 0),
                    stop=(tb == TB - 1),
                )
            rden_h = const.tile([1, HB], f16, tag=f"rd{h}", name=f"rd{h}")
            with nc.allow_low_precision(reason="1/denom fp16: rel 5e-4 << 2e-2"):
                nc.vector.reciprocal(out=rden_h[:], in_=sums_h[:])
            rdps_h = psum.tile([P, HB], f32, tag=f"rdp{h}", name=f"rdp{h}")
            nc.tensor.matmul(
                rdps_h[:], lhsT=ones_row[:], rhs=rden_h[:], start=True, stop=True
            )
            attn_h = const.tile([P, TB, HB], f32, tag=f"at{h}", name=f"at{h}")
            nc.vector.tensor_tensor(
                attn_h[:],
                em_h[:],
                rdps_h[:].unsqueeze(1).broadcast_to((P, TB, HB)),
                mybir.AluOpType.mult,
            )
            rings[h].dma_start(out=out_e[h], in_=attn_h[:])

    nc.compile()
    return nc


def _get_nc():
    if "nc" not in _STATE:
        _STATE["nc"] = _build_nc()
    return _STATE["nc"]


def _make_in_maps(query, key, mask, W, bias):
    from concourse import mybir

    f8np = mybir.dt.np(mybir.dt.float8e4)

    query = np.asarray(query, dtype=np.float32)
    key = np.asarray(key, dtype=np.float32)
    mask = np.asarray(mask, dtype=np.float32)
    W = np.asarray(W, dtype=np.float32)
    bias = np.asarray(bias, dtype=np.float32).reshape(-1)

    # wt[h, qp, qh, kc, kl] = W[kc*128 + kl, (h*4 + qh)*128 + qp]
    WT = np.ascontiguousarray(
        W.T.astype(np.float16)
        .reshape(2, QC // 2, P, KC, P)
        .transpose(0, 2, 1, 3, 4)
    )
    biasb = np.ascontiguousarray(
        np.broadcast_to(bias[:1][None, :], (P, 1)).astype(np.float32)
    )
    key16 = key.astype(np.float16)

    in_maps = []
    for i in range(NCORES):
        sh = slice(i * BS, (i + 1) * BS)
        # keyt[kc, kp, b, t] = key[b, t, kc*128 + kp]
        keyt = np.ascontiguousarray(key16[sh].transpose(2, 0, 1)).reshape(
            KC, P, BS, T
        )
        # fp16 half-split chunks: [n, 2, P, BS//2, T]
        keyh = np.ascontiguousarray(
            keyt[: N_F16 - NQT]
            .reshape(N_F16 - NQT, P, 2, BS // 2, T)
            .transpose(0, 2, 1, 3, 4)
        )
        # fp16 quarter-split chunks: [NQT, 4, P, BS//4, T]
        keyq = np.ascontiguousarray(
            keyt[N_F16 - NQT : N_F16]
            .reshape(NQT, P, 4, BS // 4, T)
            .transpose(0, 2, 1, 3, 4)
        )
        m = {
            "keyh": keyh,
            "keyq": keyq,
            "wt": WT,
            "qt": np.ascontiguousarray(
                query[sh].T.astype(np.float16).reshape(QC, P, BS).transpose(1, 0, 2)
            ),
            "maskr": np.ascontiguousarray(
                mask[sh].T.astype(np.float16).reshape(TB, P, BS).transpose(1, 0, 2)
            ),
            "biasb": biasb,
        }
        if N_FP8:
            m["key8"] = np.ascontiguousarray(
                keyt[N_F16:]
                .astype(f8np)
                .reshape(N_FP8, P, 2, BS // 2, T)
                .transpose(0, 2, 1, 3, 4)
            )
        in_maps.append(m)
    return in_maps


def _run(in_maps, **kwargs):
    from concourse.bass_utils import run_bass_kernel_spmd

    return run_bass_kernel_spmd(
        _get_nc(), in_maps, core_ids=list(range(NCORES)), **kwargs
    )


def _gather(results):
    # out[h, tp, tb, bh] -> attn[h*8 + bh, tb*128 + tp]
    return np.concatenate(
        [
            np.asarray(r["out"]).transpose(0, 3, 2, 1).reshape(BS, T)
            for r in results
        ],
        axis=0,
    )


def kernel(query, key, mask, W, bias):
    in_maps = _make_in_maps(query, key, mask, W, bias)
    res = _run(in_maps)
    return _gather(res.results)
